# revision 1
# baseline (speedup 1.0000x reference)
"""Causal multi-head self-attention with RoPE on 8 Trainium2 NeuronCores.

Problem: B=2, S=2048, D=1024, 16 heads, d_k=64, fp32.

Sharding: core c -> (batch b = c//4, head-group g = c%4 of 4 heads).
Each core computes QKV projections for its batch (fp32r matmuls), RoPE,
causal attention for its 4 heads, and a partial output projection
y_partial = attn_out_g @ Wo[:, g_cols].T.  The host sums the 4 partials
per batch (the tensor-parallel all-reduce).

Device dataflow (per core):
  - activations kept head-dim-major: qT/kT [128, 2048] per head-pair
    (partitions = 2 heads x 64 dims, free = tokens).
  - RoPE: the interleaved even/odd rotation is re-expressed as rotate-half
    by permuting Wq/Wk rows per head on the host (scores are invariant to a
    shared permutation of q/k head dims). The 1/sqrt(d_k) scale is folded
    into Wq.  The cross-half combination uses a PE permutation matmul
    (swap 32-row halves) so every vector op stays partition-aligned:
      q' = q_tilde * CS + swap(q_tilde) * SN_signed
  - scores computed transposed: S^T[tk, tq] = kT_blk.T @ qT (so exp(S^T)
    tiles feed the PV matmul as the stationary operand with no transposes).
  - softmax denominator: V gets an appended ones-column, so the PV matmul
    accumulates both attn_out (rows 0..63) and the row-sum (row 64).
    exp() runs on the scalar engine straight out of PSUM; causal masking
    multiplies the four diagonal tiles by a host-built 0/1 mask.
  - normalization: reciprocal of the rowsum broadcast across partitions via
    a K=1 matmul with a ones vector, then one elementwise multiply.  Head
    B's normalized tile is moved to partitions 64..127 with an SBUF->SBUF
    DMA (engines cannot cross partitions).
"""
import os
import sys
import numpy as np

for _p in ("/opt/trn_rl_repo", "/root/.axon_site/_ro/trn_rl_repo"):
    if _p not in sys.path and os.path.isdir(_p):
        sys.path.insert(0, _p)

D = 1024
S = 2048
NH = 16
DK = 64
HG = 256          # head-group width per core (4 heads)
TB = 512          # token block
NT = S // TB      # 4
THETA = 10000.0

_CACHE = {}


def _build_nc(reps=1):
    import concourse.tile as tile
    from concourse import bacc, mybir
    
    F32 = mybir.dt.float32
    F32R = mybir.dt.float32r
    EXP = mybir.ActivationFunctionType.Exp

    nc = bacc.Bacc("TRN2", target_bir_lowering=False)
    xt = nc.dram_tensor("xt", [D, S], F32R, kind="ExternalInput")
    wqt = nc.dram_tensor("wqt", [D, HG], F32R, kind="ExternalInput")
    wkt = nc.dram_tensor("wkt", [D, HG], F32R, kind="ExternalInput")
    wvt = nc.dram_tensor("wvt", [D, HG], F32R, kind="ExternalInput")
    wot = nc.dram_tensor("wot", [HG, D], F32R, kind="ExternalInput")
    swp = nc.dram_tensor("swp", [128, 128], F32R, kind="ExternalInput")
    cs = nc.dram_tensor("cs", [128, S], F32, kind="ExternalInput")
    sn = nc.dram_tensor("sn", [128, S], F32, kind="ExternalInput")
    yt = nc.dram_tensor("yt", [D, S], F32, kind="ExternalOutput")
    scr = nc.dram_tensor("scr", [16, TB], F32)

    from contextlib import nullcontext

    with tile.TileContext(nc) as tc:
        with tc.tile_pool(name="consts", bufs=1) as consts, \
             tc.tile_pool(name="persist", bufs=1) as persist:
            ident = consts.tile([128, 128], F32)
            ones32 = consts.tile([128, 128], F32)
            nc.vector.memset(ones32, 1.0)
            ones_sb = None
            swp_sb = consts.tile([128, 128], F32R)
            cs_sb = consts.tile([128, S], F32)
            sn_sb = consts.tile([128, S], F32)
            msk_sb = consts.tile([128, 128], F32, name="msk_sb")

            qT = [persist.tile([128, S], F32R, name=f"qT{i}") for i in range(2)]
            kT = [persist.tile([128, S], F32R, name=f"kT{i}") for i in range(2)]
            # token-major V-hat per pair: per 128-token block, cols 0:64 head A,
            # 64 ones, 65:129 head B, 129 ones
            vh = [persist.tile([128, S // 128, 130], F32R, name=f"vh{i}")
                  for i in range(2)]
            attnT = [persist.tile([128, S], F32R, name=f"attnT{i}")
                     for i in range(2)]
            for ic in range(2):
                nc.vector.tensor_copy(vh[ic][:, :, 64], ones32[:, 0:S // 128])
                nc.vector.tensor_copy(vh[ic][:, :, 129], ones32[:, 0:S // 128])

            # optional on-device repeat loop for benchmarking (reps>1); the
            # barrier keeps in-flight DMA of one iteration out of the next
            loop_cm = tc.For_i(0, reps, 1) if reps != 1 else nullcontext()
            with loop_cm:
                _phases(nc, tc, tile, mybir, locals())
                if reps != 1:
                    tc.strict_bb_all_engine_barrier()
    nc.compile()
    return nc


def _phases(nc, tc, tile, mybir, env):
    from contextlib import ExitStack
    import concourse.bass as _bass
    from concourse.masks import make_identity

    F32 = mybir.dt.float32
    F32R = mybir.dt.float32r
    EXP = mybir.ActivationFunctionType.Exp
    xt, wqt, wkt, wvt, wot = env["xt"], env["wqt"], env["wkt"], env["wvt"], env["wot"]
    yt = env["yt"]
    ident, ones_sb, swp_sb = env["ident"], env["ones_sb"], env["swp_sb"]
    cs_sb, sn_sb, msk_sb = env["cs_sb"], env["sn_sb"], env["msk_sb"]
    qT, kT, vh, attnT = env["qT"], env["kT"], env["vh"], env["attnT"]
    swp, cs, sn = env["swp"], env["cs"], env["sn"]
    scr = env["scr"]

    # Engines execute their instruction streams in program order, so cross-
    # phase overlap must be EMITTED interleaved.  We software-pipeline:
    #   A(0), [B(0) x A(1)], [B(1) x A(2)], [B(2) x A(3)], [B(3) x C(0..2)], C(3)
    # where A(t) projects token block t (attention for query block J needs
    # K/V only through block J), B(J) is query block J's attention, and C(t)
    # is the output projection for token block t.
    # PSUM tags: "a" = A-side scratch + C's accumulators (2 banks),
    # "w" = QK score tiles (2x2 banks), "pvy" = PV accumulators (2 banks).
    with tc.tile_pool(name="expw", bufs=5) as epool, \
         tc.tile_pool(name="rs", bufs=1) as rsp, \
         tc.tile_pool(name="bshift", bufs=2) as bsh, \
         tc.tile_pool(name="yst", bufs=3) as ysp, \
         tc.tile_pool(name="wo", bufs=1) as wop, \
         tc.tile_pool(name="wghts", bufs=1) as wpool, \
         tc.tile_pool(name="xts", bufs=2) as xpool, \
         tc.tile_pool(name="ropet", bufs=2) as rpool, \
         tc.tile_pool(name="pbc", bufs=2, space="PSUM") as pbc:
        # startup DMAs: first x block + weights on parallel queues
        xtv = xt.rearrange("(i p) s -> p i s", p=128)
        x_first = []
        for h in range(2):
            xh = xpool.tile([128, 4, TB], F32R, tag="x", bufs=3,
                            name=f"xf{h}")
            nc.sync.dma_start(out=xh, in_=xtv[:, 4 * h:4 * h + 4, 0:TB])
            x_first.append(xh)
        wsbs = {}
        for name, srct in (("q", wqt), ("k", wkt), ("v", wvt)):
            wsb = wpool.tile([128, 8, HG], F32R, name=f"w{name}_sb")
            nc.scalar.dma_start(
                out=wsb, in_=srct.rearrange("(i p) n -> p i n", p=128))
            wsbs[name] = wsb
        nc.scalar.dma_start(out=swp_sb, in_=swp[:])
        nc.scalar.dma_start(out=cs_sb, in_=cs[:])
        nc.scalar.dma_start(out=sn_sb, in_=sn[:])
        wo_sb = wop.tile([128, 2, D], F32R)
        nc.gpsimd.dma_start(out=wo_sb[:, 0, :], in_=wot[0:128, :])
        nc.gpsimd.dma_start(out=wo_sb[:, 1, :], in_=wot[128:256, :])
        # on-device constants (Pool engine; no DMA-issue role here)
        make_identity(nc, ident)
        nc.gpsimd.memset(msk_sb, 1.0)
        nc.gpsimd.affine_select(
            out=msk_sb, in_=msk_sb,
            compare_op=mybir.AluOpType.is_ge, fill=0.0,
            base=0, channel_multiplier=-1, pattern=[[1, 128]])

        def a_work(t):
            """Projections + RoPE + V transpose for token block t."""
            tsl = slice(t * TB, (t + 1) * TB)
            if t == 0:
                x_sb = x_first
            else:
                x_sb = []
                for h in range(2):
                    xh = xpool.tile([128, 4, TB], F32R, tag="x", bufs=3,
                                    name=f"x{t}_{h}")
                    nc.sync.dma_start(out=xh, in_=xtv[:, 4 * h:4 * h + 4, tsl])
                    x_sb.append(xh)
            for ic in range(2):
                csl = slice(ic * 128, (ic + 1) * 128)
                for name, dest in (("q", qT), ("k", kT)):
                    ps = pbc.tile([128, TB], F32, tag="a", bufs=2)
                    for i in range(8):
                        nc.tensor.matmul(ps, wsbs[name][:, i, csl],
                                         x_sb[i // 4][:, i % 4, :],
                                         start=(i == 0), stop=(i == 7))
                    yield
                    # rope: dest = qt*CS + swap(qt)*SN_signed
                    qt_sb = rpool.tile([128, TB], F32R, tag="qt", bufs=2)
                    nc.vector.tensor_copy(qt_sb, ps)
                    sw_ps = pbc.tile([128, TB], F32, tag="a", bufs=2)
                    nc.tensor.matmul(sw_ps, swp_sb, qt_sb,
                                     start=True, stop=True)
                    t1 = rpool.tile([128, TB], F32, tag="t1", bufs=2)
                    t2 = rpool.tile([128, TB], F32, tag="t2", bufs=2)
                    nc.vector.tensor_mul(t1, qt_sb, cs_sb[:, tsl])
                    nc.vector.tensor_mul(t2, sw_ps, sn_sb[:, tsl])
                    nc.vector.tensor_add(dest[ic][:, tsl], t1, t2)
                    yield
                ps = pbc.tile([128, TB], F32, tag="a", bufs=2)
                for i in range(8):
                    nc.tensor.matmul(ps, wsbs["v"][:, i, csl],
                                     x_sb[i // 4][:, i % 4, :],
                                     start=(i == 0), stop=(i == 7))
                yield
                vtmp = rpool.tile([128, TB], F32, tag="vtmp", bufs=2)
                nc.vector.tensor_copy(vtmp, ps)
                for s4 in range(4):
                    pta = pbc.tile([128, TB], F32, tag="a", bufs=2)
                    pt = pta[:, 0:128]
                    nc.tensor.transpose(pt, vtmp[:, 128 * s4:128 * (s4 + 1)],
                                        ident)
                    blk = t * 4 + s4
                    vh_dst = vh[ic][:, blk, :].rearrange(
                        "p (h c) -> p h c", h=2)
                    vh_dst = _bass.AP(
                        tensor=vh_dst.tensor, offset=vh_dst.offset,
                        ap=[vh_dst.ap[0], [65, 2], [1, 64]])
                    nc.vector.tensor_copy(
                        vh_dst, pt.rearrange("p (h c) -> p h c", h=2))
                    if s4 % 2 == 1:
                        yield

        def b_work(J):
            """Attention for query block J (both head pairs)."""
            nblk = 4 * J + 4
            Jsl = slice(J * TB, (J + 1) * TB)
            for ic in range(2):
                pvA = pbc.tile([128, TB], F32, tag="pvy", bufs=2)
                pvB = pbc.tile([128, TB], F32, tag="pvy", bufs=2)
                for i in range(nblk):
                    tkb = slice(128 * i, 128 * (i + 1))
                    wps = pbc.tile([128, 2, TB], F32, tag="w", bufs=2)
                    nc.tensor.matmul(wps[:, 0, :], kT[ic][0:64, tkb],
                                     qT[ic][0:64, Jsl],
                                     start=True, stop=True)
                    nc.tensor.matmul(wps[:, 1, :], kT[ic][64:128, tkb],
                                     qT[ic][64:128, Jsl],
                                     start=True, stop=True)
                    ew = epool.tile([128, 2, TB], F32R, tag="e")
                    ii = i - 4 * J
                    if ii <= 0:
                        nc.scalar.activation(ew, wps, EXP)
                    else:
                        # tq < 128*ii is fully masked: zero-fill, exp the rest
                        lo = 128 * ii
                        nc.vector.memset(ew[:, :, 0:lo].bitcast(F32), 0.0)
                        nc.scalar.activation(ew[:, :, lo:TB],
                                             wps[:, :, lo:TB], EXP)
                    if ii >= 0:
                        tw = slice(128 * ii, 128 * ii + 128)
                        nc.gpsimd.tensor_mul(ew[:, 0, tw], ew[:, 0, tw],
                                             msk_sb)
                        nc.gpsimd.tensor_mul(ew[:, 1, tw], ew[:, 1, tw],
                                             msk_sb)
                    nc.tensor.matmul(pvA[0:65, :], vh[ic][:, i, 0:65],
                                     ew[:, 0, :], start=(i == 0),
                                     stop=(i == nblk - 1))
                    nc.tensor.matmul(pvB[0:65, :], vh[ic][:, i, 65:130],
                                     ew[:, 1, :], start=(i == 0),
                                     stop=(i == nblk - 1))
                    yield
                pvAs = rsp.tile([65, TB], F32, tag="pvs", bufs=2)
                pvBs = rsp.tile([65, TB], F32, tag="pvs", bufs=2)
                nc.vector.tensor_copy(pvAs, pvA[0:65, :])
                nc.vector.tensor_copy(pvBs, pvB[0:65, :])
                rA = rsp.tile([65, TB], F32, tag="r", bufs=2)
                rB = rsp.tile([65, TB], F32, tag="r", bufs=2)
                nc.vector.reciprocal(rA[64:65, :], pvAs[64:65, :])
                nc.vector.reciprocal(rB[64:65, :], pvBs[64:65, :])
                # partition-broadcast the reciprocal rows through DRAM
                row = (ic * 4 + J) * 2
                nc.sync.dma_start(out=scr[row:row + 1, :], in_=rA[64:65, :])
                nc.sync.dma_start(out=scr[row + 1:row + 2, :],
                                  in_=rB[64:65, :])
                rbcA = rsp.tile([64, TB], F32, tag="rbc", bufs=2)
                rbcB = rsp.tile([64, TB], F32, tag="rbc", bufs=2)
                nc.sync.dma_start(
                    out=rbcA, in_=scr[row:row + 1, :].partition_broadcast(64))
                nc.sync.dma_start(
                    out=rbcB,
                    in_=scr[row + 1:row + 2, :].partition_broadcast(64))
                yield
                nc.vector.tensor_mul(attnT[ic][0:64, Jsl], pvAs[0:64, :],
                                     rbcA)
                tmpB = bsh.tile([64, TB], F32R, tag="tmpB")
                nc.vector.tensor_mul(tmpB, pvBs[0:64, :], rbcB)
                nc.sync.dma_start(out=attnT[ic][64:128, Jsl], in_=tmpB)
                yield

        def c_work(t):
            """Output projection for token block t."""
            tsl = slice(t * TB, (t + 1) * TB)
            for j in range(8):
                jsl = slice(128 * j, 128 * (j + 1))
                yp = pbc.tile([128, TB], F32, tag="a", bufs=2)
                for ic in range(2):
                    nc.tensor.matmul(yp, wo_sb[:, ic, jsl],
                                     attnT[ic][:, tsl],
                                     start=(ic == 0), stop=(ic == 1))
                ys = ysp.tile([128, TB], F32, tag="ys")
                if j % 2 == 0:
                    nc.vector.tensor_copy(ys, yp)
                else:
                    nc.scalar.copy(ys, yp)
                nc.sync.dma_start(out=yt[jsl, tsl], in_=ys)
                yield

        def interleave(primary, secondary, ratio=(1, 1)):
            """Alternate emission: ratio[0] primary units per ratio[1]
            secondary units; drains whichever stream remains."""
            np_, ns = ratio
            pa, sa = True, True
            while pa or sa:
                for _ in range(np_):
                    pa = next(primary, _END) is not _END if pa else False
                for _ in range(ns):
                    sa = next(secondary, _END) is not _END if sa else False

        _END = object()

        def chain(*gens):
            for g in gens:
                yield from g

        for _ in a_work(0):
            pass
        interleave(b_work(0), a_work(1))
        interleave(b_work(1), a_work(2))
        interleave(b_work(2), a_work(3))
        # pace C(0..2) across all of B(3) (~36 vs 24 units -> 3:2)
        interleave(b_work(3), chain(c_work(0), c_work(1), c_work(2)),
                   ratio=(2, 1))
        for _ in c_work(3):
            pass


def _host_prep(in_features, token_positions, Wq, Wk, Wv, Wo):
    X = np.ascontiguousarray(np.asarray(in_features, dtype=np.float32))
    pos = np.asarray(token_positions)
    Wq = np.asarray(Wq, dtype=np.float32)
    Wk = np.asarray(Wk, dtype=np.float32)
    Wv = np.asarray(Wv, dtype=np.float32)
    Wo = np.asarray(Wo, dtype=np.float32)

    freq = 1.0 / np.power(np.float32(THETA),
                          np.arange(0, DK, 2, dtype=np.float32) / DK)
    freqs = np.outer(pos.astype(np.float32), freq)      # [S, 32]
    CS = np.tile(np.cos(freqs).T.astype(np.float32), (4, 1))  # [128, S]
    sinT = np.sin(freqs).T.astype(np.float32)           # [32, S]
    SN = np.concatenate([-sinT, sinT, -sinT, sinT], axis=0)   # signed [128, S]

    # swap matrix: exchanges 32-row halves within each 64-row head block
    swap = np.arange(128)
    swap = np.where(swap % 64 < 32, swap + 32, swap - 32)
    SWP = np.zeros((128, 128), np.float32)
    SWP[swap, np.arange(128)] = 1.0



    perm = np.concatenate([np.arange(0, DK, 2), np.arange(1, DK, 2)])

    def prep_qk(W, scale):
        out = {}
        for g in range(4):
            rows = [W[h * DK:(h + 1) * DK][perm] * scale
                    for h in range(4 * g, 4 * g + 4)]
            out[g] = np.ascontiguousarray(np.concatenate(rows, axis=0).T)
        return out

    wqts = prep_qk(Wq, np.float32(1.0 / 8.0))
    wkts = prep_qk(Wk, np.float32(1.0))
    wvts = {g: np.ascontiguousarray(Wv[HG * g:HG * (g + 1)].T) for g in range(4)}
    wots = {g: np.ascontiguousarray(Wo[:, HG * g:HG * (g + 1)].T)
            for g in range(4)}
    xts = {b: np.ascontiguousarray(X[b].T) for b in range(2)}

    in_maps = []
    for c in range(8):
        b, g = c // 4, c % 4
        in_maps.append({
            "xt": xts[b], "wqt": wqts[g], "wkt": wkts[g], "wvt": wvts[g],
            "wot": wots[g], "swp": SWP, "cs": CS, "sn": SN,
        })
    return in_maps


def kernel(in_features, token_positions, Wq, Wk, Wv, Wo):
    from concourse.bass_utils import run_bass_kernel_spmd

    if "nc" not in _CACHE:
        _CACHE["nc"] = _build_nc()
    nc = _CACHE["nc"]

    in_maps = _host_prep(in_features, token_positions, Wq, Wk, Wv, Wo)
    res = run_bass_kernel_spmd(nc, in_maps, list(range(8)))

    B = np.asarray(in_features).shape[0]
    y = np.zeros((B, S, D), np.float32)
    for c in range(8):
        b = c // 4
        y[b] += res.results[c]["yt"].T
    return y

